# revision 12
# baseline (speedup 1.0000x reference)
"""Causal multi-head self-attention with RoPE on 8 Trainium2 NeuronCores.

Sharding: data-parallel over batch (B=4 -> 2 cores per batch) x tensor-parallel
over heads (16 heads -> 8 per core). Each core computes q/k/v projections for
its 8 heads, RoPE, causal attention, and a partial o_proj; the host sums the
two partial o_proj outputs per batch.

v2: single fused pipeline. The ACT engine's exp is the attention bottleneck
(~1.15us per 128x1024 score tile), so projection / o_proj matmul chains are
emitted as paced "filler" units between attention matmuls: the PE never waits
for exp. All matmul operands are bf16 (inputs cast on host), PSUM stays f32;
exp writes bf16 e-tiles directly. Per-core layout as v1:
  - x^T and pre-transposed weight shards DMA'd in; Q^T/K^T head-major [dk,s];
    scores computed transposed [s_k,s_q]; ones-column-augmented V gives the
    softmax denominator for free in the AV accumulation.
  - Scores for the two heads of a 128-row chunk packed into PE row-groups via
    tile_position; one full-width exp covers both.
  - RoPE via evens/odds dk permutation (host-folded) + P_swap matmul;
    cos/sin tables built on device with Cody-Waite range reduction.
  - Diagonal-block causal mask via gpsimd affine_select; diag key-tiles run
    first in each accumulation group.
"""

import sys

sys.path.insert(0, "/opt/trn_rl_repo")

import numpy as np
import ml_dtypes

import concourse.bass as bass
import concourse.tile as tile
from concourse import bacc, mybir
from concourse.bass_utils import run_bass_kernel_spmd
from concourse.masks import make_identity

B, S, D, H = 4, 2048, 1024, 16
DK = D // H            # 64
HPC = H // 2           # 8 heads per core
DPC = HPC * DK         # 512 head dims per core
N_CORES = 8
HALF = DK // 2         # 32 rotary pairs
THETA = 10000.0

AF = mybir.ActivationFunctionType
F32 = mybir.dt.float32
F32R = mybir.dt.float32r
BF16 = mybir.dt.bfloat16
I32 = mybir.dt.int32

TWO_PI = 2.0 * np.pi
_CW_C1 = 6.28125
_CW_C2 = float(np.float32(9.67025756835937500e-4))
_CW_C3 = float(TWO_PI - _CW_C1 - np.float32(9.67025756835937500e-4))

# pacing model (ns) for the filler scheduler
_EXP_NS = 1210.0
_YIELD_NS = 450.0


def _mm_ns(n):
    return n / 2.4 + 90.0


def _build_program(debug=False):
    nc = bacc.Bacc("TRN2", target_bir_lowering=False, debug=False)

    xT = nc.dram_tensor("xT", [D, S], BF16, kind="ExternalInput").ap()
    wqT = nc.dram_tensor("wqT", [D, DPC], BF16, kind="ExternalInput").ap()
    wkT = nc.dram_tensor("wkT", [D, DPC], BF16, kind="ExternalInput").ap()
    wvT = nc.dram_tensor("wvT", [D, DPC], BF16, kind="ExternalInput").ap()
    woT = nc.dram_tensor("woT", [DPC, D], BF16, kind="ExternalInput").ap()
    pos = nc.dram_tensor("pos", [S], I32, kind="ExternalInput").ap()
    invf_in = nc.dram_tensor("invf", [HALF], F32, kind="ExternalInput").ap()
    y = nc.dram_tensor("y", [S, D], F32, kind="ExternalOutput").ap()

    dbg = None
    if debug:
        dbg = {
            "cs_dump": nc.dram_tensor("cs_dump", [2, 128, S], F32, kind="ExternalOutput").ap(),
            "qk_dump": nc.dram_tensor("qk_dump", [128, 8, S], BF16, kind="ExternalOutput").ap(),
            "vp_dump": nc.dram_tensor("vp_dump", [128, S // 128, HPC * (DK + 1)], BF16, kind="ExternalOutput").ap(),
            "heads_dump": nc.dram_tensor("heads_dump", [128, DPC // 128, S], BF16, kind="ExternalOutput").ap(),
        }

    with tile.TileContext(nc) as tc:
        _emit(nc, tc, xT, wqT, wkT, wvT, woT, pos, invf_in, y, dbg)

    nc.compile()
    return nc


def _emit(nc, tc, xT, wqT, wkT, wvT, woT, pos, invf_in, y, dbg=None):
    import contextlib

    ctx = contextlib.ExitStack()
    with ctx:
        persist = ctx.enter_context(tc.tile_pool(name="persist", bufs=1))
        identity = persist.tile([128, 128], F32)
        make_identity(nc, identity)
        # P_swap: swap 32-row blocks within each 64-block
        p_swap = persist.tile([128, 128], F32R)
        for blk in range(4):
            src_row = (blk ^ 1) * 32
            nc.sync.dma_start(
                out=p_swap[blk * 32:(blk + 1) * 32, :],
                in_=identity.bitcast(F32R)[src_row:src_row + 32, :],
            )
        ones_col = persist.tile([128, 1], BF16)
        nc.vector.memset(ones_col, 1.0)

        # ---- persistent tensors ----
        cs_pool = ctx.enter_context(tc.tile_pool(name="cs", bufs=1))
        cbig = cs_pool.tile([128, S], F32)
        sbig = cs_pool.tile([128, S], F32)
        qkT = ctx.enter_context(tc.tile_pool(name="qkT", bufs=1)).tile(
            [128, 8, S], BF16)
        vp = ctx.enter_context(tc.tile_pool(name="vp", bufs=1)).tile(
            [128, S // 128, HPC * (DK + 1)], BF16)
        vph = vp.rearrange("p s (h c) -> p s h c", h=HPC)
        heads_t = ctx.enter_context(tc.tile_pool(name="heads", bufs=1)).tile(
            [128, DPC // 128, S], BF16)
        wpool = ctx.enter_context(tc.tile_pool(name="w", bufs=1))
        w_qk = wpool.tile([128, 2, D // 128, DPC], BF16)
        wv_t = wpool.tile([128, D // 128, DPC], BF16)
        wo_t = wpool.tile([128, DPC // 128, D], BF16)

        xts_pool = ctx.enter_context(tc.tile_pool(name="xts", bufs=2))
        e_pool = ctx.enter_context(tc.tile_pool(name="e", bufs=6))
        tmp = ctx.enter_context(tc.tile_pool(name="tmp", bufs=2))
        norm_pool = ctx.enter_context(tc.tile_pool(name="norm", bufs=2))
        ysb_pool = ctx.enter_context(tc.tile_pool(name="ysb", bufs=2))

        ps_s = ctx.enter_context(tc.tile_pool(name="ps_s", bufs=2, space="PSUM"))
        ps_o = ctx.enter_context(tc.tile_pool(name="ps_o", bufs=2, space="PSUM"))
        ps_j = ctx.enter_context(tc.tile_pool(name="ps_j", bufs=2, space="PSUM"))

        # ---- DMA prefetch ----
        tblp = tc.alloc_tile_pool(name="tbl", bufs=5)
        posi = tblp.tile([1, S], I32, name="posi", bufs=1)
        nc.gpsimd.dma_start(out=posi, in_=pos.unsqueeze(0))
        invf = tblp.tile([1, HALF], F32, name="invf", bufs=1)
        nc.gpsimd.dma_start(out=invf, in_=invf_in.unsqueeze(0))
        for dc in range(D // 128):
            nc.sync.dma_start(out=w_qk[:, 0, dc, :],
                              in_=wqT[dc * 128:(dc + 1) * 128, :])
            nc.scalar.dma_start(out=w_qk[:, 1, dc, :],
                                in_=wkT[dc * 128:(dc + 1) * 128, :])
        for dc in range(D // 128):
            nc.gpsimd.dma_start(out=wv_t[:, dc, :],
                                in_=wvT[dc * 128:(dc + 1) * 128, :])
        for dc in range(DPC // 128):
            nc.gpsimd.dma_start(out=wo_t[:, dc, :],
                                in_=woT[dc * 128:(dc + 1) * 128, :])

        xts_tiles = {}

        def load_x(sc, engs):
            t = xts_pool.tile([128, D // 128, 512], BF16, name="xts")
            for dc in range(D // 128):
                engs[dc % len(engs)].dma_start(
                    out=t[:, dc, :],
                    in_=xT[dc * 128:(dc + 1) * 128, bass.ts(sc, 512)])
            xts_tiles[sc] = t

        load_x(0, (nc.sync, nc.scalar))
        load_x(1, (nc.sync, nc.scalar))

        # ---- rotary tables (ACT/DVE; overlaps weight DMA + first proj) ----
        # processed in two 1024-col chunks to bound SBUF; cody/k_i read the
        # angle product straight out of PSUM
        for hf in range(2):
            sl = bass.ts(hf, 1024)
            posf = tblp.tile([1, 1024], F32, name="posf", tag="tbl8")
            nc.vector.tensor_copy(posf, posi[:, sl])
            ang_ps = ps_s.tile([HALF, 1024], F32, name="sc_t", tag="sc_t")
            for j in range(2):
                nc.tensor.matmul(
                    ang_ps[:, j * 512:(j + 1) * 512], invf,
                    posf[:, j * 512:(j + 1) * 512], start=True, stop=True)
            k_i = tblp.tile([HALF, 1024], I32, name="k_i", tag="tbl8")
            nc.scalar.activation(k_i, ang_ps, AF.Copy, scale=float(1.0 / TWO_PI))
            k_f = tblp.tile([HALF, 1024], F32, name="k_f", tag="tbl8")
            nc.vector.tensor_copy(k_f, k_i)
            ang_red = tblp.tile([HALF, 1024], F32, name="ang_red", tag="tbl8")
            nc.vector.cody_waite_cascade(ang_red, ang_ps, k_f,
                                         _CW_C1, _CW_C2, _CW_C3)
            sin_arg = tblp.tile([HALF, 1024], F32, name="sin_arg", tag="tbl8")
            cos_arg = tblp.tile([HALF, 1024], F32, name="cos_arg", tag="tbl8")
            nc.vector.add_range_wrap(sin_arg, ang_red, 0.0, float(np.pi), TWO_PI)
            nc.vector.add_range_wrap(cos_arg, ang_red, float(np.pi / 2),
                                     float(np.pi), TWO_PI)
            nc.scalar.activation(cbig[0:HALF, sl], cos_arg, AF.Sin)
            s_pos = tblp.tile([HALF, 1024], F32, name="s_pos", tag="tbl8")
            nc.scalar.activation(s_pos, sin_arg, AF.Sin)
            nc.vector.tensor_scalar_mul(sbig[0:HALF, sl], s_pos, -1.0)
            nc.gpsimd.dma_start(out=sbig[HALF:2 * HALF, sl], in_=s_pos)
        nc.gpsimd.dma_start(out=cbig[HALF:2 * HALF, :], in_=cbig[0:HALF, :])
        nc.gpsimd.dma_start(out=cbig[64:128, :], in_=cbig[0:64, :])
        nc.gpsimd.dma_start(out=sbig[64:128, :], in_=sbig[0:64, :])
        tblp.release()

        # ones column of the V layout (denominator trick)
        nc.scalar.copy(vph[:, :, :, DK:DK + 1],
                       ones_col.to_broadcast((128, S // 128, HPC, 1)))

        # ---- emission units (generators; one `yield` ~ 2 matmuls of PE) ----
        def gen_qk_unit(sc, qk, et, dense=False):
            ssl = bass.ts(sc, 512)
            if dense:
                p_t = ps_s.tile([128, 512], F32, name="p_t", tag="sc_t")
            else:
                p_t = ps_j.tile([128, 512], F32, name="p_t", tag="j")
            xts = xts_tiles[sc]
            for dc in range(D // 128):
                nc.tensor.matmul(
                    p_t, w_qk[:, qk, dc, et * 128:(et + 1) * 128], xts[:, dc, :],
                    start=(dc == 0), stop=(dc == D // 128 - 1))
                if dc % 2 == 1 and dc < 7:
                    yield
            qt_sb = tmp.tile([128, 512], F32R, name="qt_sb")
            nc.vector.tensor_copy(qt_sb, p_t)
            yield
            sw = ps_j.tile([128, 512], F32, name="sw", tag="j")
            nc.tensor.matmul(sw, p_swap, qt_sb, start=True, stop=True)
            g1 = tmp.tile([128, 512], F32, name="g1")
            nc.gpsimd.tensor_mul(g1, qt_sb.bitcast(F32), cbig[:, ssl])
            d1 = tmp.tile([128, 512], F32, name="d1")
            nc.vector.tensor_mul(d1, sw, sbig[:, ssl])
            nc.vector.tensor_add(qkT[:, qk * 4 + et, ssl], g1, d1)
            yield

        def gen_v_unit(sc, st4, dense=False):
            if dense:
                p_t = ps_s.tile([128, 512], F32, name="pv_t", tag="sc_t")
            else:
                p_t = ps_j.tile([128, 512], F32, name="pv_t", tag="j")
            xts = xts_tiles[sc]
            for dc in range(D // 128):
                nc.tensor.matmul(
                    p_t, xts[:, dc, st4 * 128:(st4 + 1) * 128], wv_t[:, dc, :],
                    start=(dc == 0), stop=(dc == D // 128 - 1))
                if dc % 2 == 1 and dc < 7:
                    yield
            nc.vector.tensor_copy(
                vph[:, sc * 4 + st4, :, 0:DK],
                p_t.rearrange("p (h c) -> p h c", h=HPC))
            yield

        def gen_oproj_unit(qc, st4, nb):
            st = qc * 4 + st4
            py = ps_j.tile([128, 512], F32, name="py", tag="j")
            for dc in range(DPC // 128):
                nc.tensor.matmul(
                    py, heads_t[:, dc, st * 128:(st + 1) * 128],
                    wo_t[:, dc, bass.ts(nb, 512)],
                    start=(dc == 0), stop=(dc == DPC // 128 - 1))
                if dc == 1:
                    yield
            y_sb = ysb_pool.tile([128, 512], F32, name="y_sb")
            nc.vector.tensor_copy(y_sb, py)
            nc.sync.dma_start(
                out=y[st * 128:(st + 1) * 128, bass.ts(nb, 512)], in_=y_sb)
            yield

        # ---- filler queue ----
        filler_q = []   # entries: (key, generator)

        def push_proj(sc):
            for et in range(4):
                filler_q.append((("p", sc), gen_qk_unit(sc, 1, et)))
                filler_q.append((("p", sc), gen_qk_unit(sc, 0, et)))
            for st4 in range(4):
                filler_q.append((("p", sc), gen_v_unit(sc, st4)))

        def push_oproj(qc):
            for st4 in range(4):
                for nb in range(D // 512):
                    filler_q.append((("o", qc), gen_oproj_unit(qc, st4, nb)))

        def pull(n):
            got = 0
            while got < n and filler_q:
                try:
                    next(filler_q[0][1])
                    got += 1
                except StopIteration:
                    filler_q.pop(0)
            return got

        def drain_key(key):
            while filler_q and any(k == key for k, _ in filler_q):
                try:
                    next(filler_q[0][1])
                except StopIteration:
                    filler_q.pop(0)

        # ---- attention for one 512-query chunk ----
        def attn(qc):
            qsl = bass.ts(qc, 512)
            n_kt = 4 * qc + 4
            kt_order = list(range(4 * qc, 4 * qc + 4)) + list(range(0, 4 * qc))
            act_t, pe_t = 0.0, 0.0
            for hp in range(HPC // 2):
                hA, hB = 2 * hp, 2 * hp + 1
                o_ts = [ps_o.tile([DK + 1, 512], F32, name=f"o{ab}", tag="o")
                        for ab in "AB"]

                def emit_scores(kt):
                    diag = (kt // 4 == qc)
                    co = 128 * (kt % 4) if diag else 0
                    n = 512 - co
                    ktsl = bass.ts(kt, 128)
                    q_ap = bass.ds(qc * 512 + co, n)
                    sc_t = ps_s.tile([128, 1024], F32, name="sc_t", tag="sc_t")
                    for i, (ro, tp) in enumerate(((0, (0, 0)), (64, (64, 0)))):
                        nc.tensor.matmul(
                            sc_t[:, i * 512:i * 512 + n],
                            qkT[ro:ro + 64, 4 + hp, ktsl],
                            qkT[ro:ro + 64, hp, q_ap],
                            start=True, stop=True, tile_position=tp)
                    e_t = e_pool.tile([128, 1024], BF16, name="e_t")
                    nc.scalar.activation(e_t, sc_t, AF.Exp,
                                         scale=float(1.0 / np.sqrt(DK)))
                    if diag:
                        for i in range(2):
                            nc.gpsimd.affine_select(
                                out=e_t[:, i * 512:i * 512 + 128],
                                in_=e_t[:, i * 512:i * 512 + 128],
                                pattern=[[1, 128]], base=0,
                                channel_multiplier=-1,
                                compare_op=mybir.AluOpType.is_ge, fill=0.0)
                    return kt, e_t, n

                def emit_av(kt, e_t, n, start, stop):
                    co = 512 - n
                    for i, h in enumerate((hA, hB)):
                        nc.tensor.matmul(
                            o_ts[i][:, co:512],
                            vp[:, kt, h * (DK + 1):(h + 1) * (DK + 1)],
                            e_t[:, i * 512:i * 512 + n],
                            start=start, stop=stop)

                prev = None
                for i, kt in enumerate(kt_order):
                    cur = emit_scores(kt)
                    act_t += _EXP_NS
                    pe_t += _mm_ns(cur[2])
                    while pe_t < act_t and filler_q:
                        pe_t += _YIELD_NS * max(1, pull(1))
                        if not filler_q:
                            break
                    if prev is not None:
                        emit_av(prev[0], prev[1], prev[2],
                                start=(i == 1), stop=False)
                        pe_t += 2 * _mm_ns(prev[2])
                    prev = cur
                emit_av(prev[0], prev[1], prev[2],
                        start=(n_kt == 1), stop=True)
                pe_t += 2 * _mm_ns(prev[2])

                # normalize: o / denominator, per head, straight out of PSUM
                dsbA = norm_pool.tile([1, 512], F32, name="dsbA", bufs=1)
                dsbB = norm_pool.tile([1, 512], F32, name="dsbB", bufs=1)
                nc.vector.tensor_copy(dsbA, o_ts[0][DK:DK + 1, :])
                nc.vector.tensor_copy(dsbB, o_ts[1][DK:DK + 1, :])
                recipA = norm_pool.tile([1, 512], F32, name="recipA", bufs=1)
                recipB = norm_pool.tile([1, 512], F32, name="recipB", bufs=1)
                nc.vector.reciprocal_approx_fast(recipA, dsbA)
                nc.vector.reciprocal_approx_fast(recipB, dsbB)
                rbA = norm_pool.tile([DK, 512], F32, name="rbA")
                rbB = norm_pool.tile([DK, 512], F32, name="rbB")
                nc.gpsimd.partition_broadcast(rbA, recipA)
                nc.gpsimd.partition_broadcast(rbB, recipB)
                nc.vector.tensor_mul(heads_t[0:DK, hp, qsl],
                                     o_ts[0][0:DK, :], rbA)
                hnB = norm_pool.tile([DK, 512], BF16, name="hnB")
                nc.vector.tensor_mul(hnB, o_ts[1][0:DK, :], rbB)
                nc.gpsimd.dma_start(out=heads_t[DK:2 * DK, hp, qsl], in_=hnB)

        # ---- fused schedule ----
        # dense proj(0) through the wide ps_s slots (attention not live yet)
        for et in range(4):
            for g in (gen_qk_unit(0, 1, et, dense=True),
                      gen_qk_unit(0, 0, et, dense=True)):
                for _ in g:
                    pass
        for st4 in range(4):
            for _ in gen_v_unit(0, st4, dense=True):
                pass

        push_proj(1)
        attn(0)
        push_proj(2)
        load_x(2, (nc.sync, nc.sync))
        drain_key(("p", 1))
        attn(1)
        push_proj(3)
        push_oproj(0)
        load_x(3, (nc.sync, nc.sync))
        drain_key(("p", 2))
        attn(2)
        push_oproj(1)
        push_oproj(2)
        drain_key(("p", 3))
        attn(3)
        push_oproj(3)
        while filler_q:
            pull(1000)

        if dbg is not None:
            nc.sync.dma_start(out=dbg["cs_dump"][0], in_=cbig)
            nc.sync.dma_start(out=dbg["cs_dump"][1], in_=sbig)
            nc.sync.dma_start(out=dbg["qk_dump"], in_=qkT)
            nc.sync.dma_start(out=dbg["vp_dump"], in_=vp)
            nc.sync.dma_start(out=dbg["heads_dump"], in_=heads_t)


def _host_inv_freq():
    v = 1.0 / (THETA ** (np.arange(HALF, dtype=np.float64) * 2.0 / DK))
    return v.astype(np.float32)


_program_cache = None


def _get_program():
    global _program_cache
    if _program_cache is None:
        _program_cache = _build_program()
    return _program_cache


# dk permutation: evens then odds within each head's 64 dims
_PERM64 = np.concatenate([np.arange(0, DK, 2), np.arange(1, DK, 2)])


def _make_in_maps(x, Wq, Wk, Wv, Wo, pos_np):
    bf = ml_dtypes.bfloat16
    invf_np = _host_inv_freq()
    in_maps = []
    for c in range(N_CORES):
        b, hg = c // 2, c % 2
        rows = hg * DPC + np.concatenate(
            [h * DK + _PERM64 for h in range(HPC)])
        in_maps.append({
            "xT": np.ascontiguousarray(x[b].T).astype(bf),
            "wqT": np.ascontiguousarray(Wq[rows, :].T).astype(bf),
            "wkT": np.ascontiguousarray(Wk[rows, :].T).astype(bf),
            "wvT": np.ascontiguousarray(Wv[hg * DPC:(hg + 1) * DPC, :].T).astype(bf),
            "woT": np.ascontiguousarray(Wo[:, hg * DPC:(hg + 1) * DPC].T).astype(bf),
            "pos": pos_np,
            "invf": invf_np,
        })
    return in_maps


def kernel(x, Wq, Wk, Wv, Wo, token_positions):
    x = np.asarray(x, dtype=np.float32)
    Wq = np.asarray(Wq, dtype=np.float32)
    Wk = np.asarray(Wk, dtype=np.float32)
    Wv = np.asarray(Wv, dtype=np.float32)
    Wo = np.asarray(Wo, dtype=np.float32)
    pos_np = np.ascontiguousarray(np.asarray(token_positions, dtype=np.int32))

    nc = _get_program()
    in_maps = _make_in_maps(x, Wq, Wk, Wv, Wo, pos_np)
    res = run_bass_kernel_spmd(nc, in_maps, list(range(N_CORES)))
    out = np.empty((B, S, D), dtype=np.float32)
    for b in range(B):
        out[b] = res.results[2 * b]["y"] + res.results[2 * b + 1]["y"]
    return out
